# revision 41
# baseline (speedup 1.0000x reference)
"""Discounted-return scan + normalize, distributed over 8 TRN2 NeuronCores.

Problem: y_i = r_i + 0.99*y_{i+1} (suffix scan over T=2**25 rewards), then
(y - mean) / (std + eps).

Strategy:
  - Host reverses rewards so the device runs a plain forward scan
    (state = g*state + s_t) along the SBUF free dimension via the
    TensorTensorScanArith ISA op on the Vector engine.
  - The T axis is sharded 8 ways; each core's shard is further split across
    128 SBUF partitions. Instead of propagating carries between partitions /
    cores, every partition scans a W-element "burn-in" prefix (overlapping
    reads): with g=0.99, a carry's influence after W=2048 steps is ~1e-9
    relative — far below the f32 resolution of the result, so the output
    matches the exact scan.
  - mean/std: per-chunk accumulated sum / sum-of-squares (ScalarE
    activation accumulate; last chunk's sum on DVE) feed one tiny 8-core
    AllGather; every core then redundantly computes the global scale/shift
    and normalizes in place. A dummy AllGather early in the kernel absorbs
    the collective firmware cold-start while the scan chain runs.
"""

import os
import sys

import numpy as np

for _p in ("/opt/trn_rl_repo", "/root/.axon_site/_ro/trn_rl_repo"):
    if os.path.isdir(_p) and _p not in sys.path:
        sys.path.insert(0, _p)

DISCOUNT = 0.99
EPS = 0.0001
T = 33554432  # 2**25
N_CORES = 8
P = 128  # SBUF partitions


def _build_nc(C, W, F):
    """Build the per-core Bass graph.

    C: elements per core (excluding burn-in), multiple of P*F
    W: burn-in prefix length per partition (also the first chunk size)
    F: main scan chunk size (columns)
    """
    import concourse.bacc as bacc
    import concourse.bass as bass
    import concourse.mybir as mybir
    from concourse import tile

    fp32 = mybir.dt.float32
    Alu = mybir.AluOpType
    Act = mybir.ActivationFunctionType
    Axis = mybir.AxisListType

    L = C // P  # valid columns per partition
    R = L + W  # total row length
    assert L % F == 0

    # chunk widths: a graduated ramp so the scan chain starts as soon as
    # the first small DMA lands and never outruns the (sequential) DMA
    # completions; F-sized steady state; last chunks halved to shorten the
    # post-scan stats tail. DMA-in uses the same chunking, all on ONE
    # HWDGE ring so completions arrive in order.
    widths = [W // 2, W - W // 2]
    rem = R - W
    ramp = W
    while ramp < F and rem - ramp >= F:
        widths.append(ramp)
        rem -= ramp
        ramp *= 2
    tail = min(1024, max(16, F // 4))
    while rem > 5 * tail:
        widths.append(min(F, rem - 4 * tail))
        rem -= widths[-1]
    while rem:
        widths.append(min(tail, rem))
        rem -= widths[-1]
    scan_chunks = []
    c = 0
    for w in widths:
        scan_chunks.append((c, w))
        c += w
    assert c == R
    n_burn = 2  # first two chunks are burn-in
    valid_chunks = scan_chunks[n_burn:]
    NV = len(valid_chunks)
    dma_chunks = scan_chunks

    nc = bacc.Bacc(
        "TRN2",
        target_bir_lowering=False,
        debug=False,
        enable_asserts=True,
        num_devices=N_CORES,
    )

    s_ext = nc.dram_tensor("s", [C + W], fp32, kind="ExternalInput")
    out_ext = nc.dram_tensor("out", [C], fp32, kind="ExternalOutput")
    cc_in = nc.dram_tensor("cc_in", [1, 2], fp32)
    cc_out = nc.dram_tensor("cc_out", [N_CORES, 2], fp32)
    warm_in = nc.dram_tensor("warm_in", [1, 2], fp32)
    warm_out = nc.dram_tensor("warm_out", [N_CORES, 2], fp32)

    inv_T = 1.0 / float(C * N_CORES)
    rg = [list(range(N_CORES))]

    with tile.TileContext(nc) as tc:
        with (
            tc.tile_pool(name="main", bufs=1) as main,
            tc.tile_pool(name="small", bufs=1) as small,
        ):
            maxw = max(w for w in widths)
            resident = main.tile([P, R], fp32)
            scratch_sq = main.tile([P, maxw], fp32)
            scratch_sum = main.tile([P, maxw], fp32)
            scratch_dve = main.tile([P, maxw], fp32)

            g_tile = small.tile([P, 1], fp32)
            scol = small.tile([P, NV], fp32)
            qcol = small.tile([P, NV], fp32)
            pay = small.tile([P, 2], fp32)
            cc_sb = small.tile([1, 2], fp32)
            gath = small.tile([P, 2 * N_CORES], fp32)
            m2 = small.tile([P, 2], fp32)
            var = small.tile([P, 1], fp32)
            std = small.tile([P, 1], fp32)
            inv = small.tile([P, 1], fp32)
            shiftp = small.tile([P, 1], fp32)

            nc.vector.memset(g_tile[:, :], DISCOUNT)
            nc.vector.memset(pay[:, :], 0.0)
            # warm the CC firmware path while the scan chain runs; SWDGE
            # queue so the big in-DMAs on the sync ring can't delay it.
            nc.gpsimd.dma_start(warm_in.ap(), pay[0:1, :])
            nc.gpsimd.collective_compute(
                "AllGather",
                Alu.bypass,
                replica_groups=rg,
                ins=[warm_in.ap().opt()],
                outs=[warm_out.ap().opt()],
            )
            # load the sqrt activation table before it's on the critical path
            nc.scalar.activation(std[:, :], g_tile[:, 0:1], Act.Sqrt)

            # ---- DMA in: one ring, in order, so completions are sequential ----
            for c0, cw in dma_chunks:
                src = bass.AP(s_ext, c0, [[L, P], [1, cw]])
                nc.sync.dma_start(resident[:, c0 : c0 + cw], src)

            # ---- chained scans + per-chunk stats ----
            last = len(scan_chunks) - 1
            for t, (c0, cw) in enumerate(scan_chunks):
                dst = resident[:, c0 : c0 + cw]
                initial = 0.0 if t == 0 else resident[:, c0 - 1 : c0]
                nc.vector.tensor_tensor_scan(
                    dst,
                    g_tile[:, 0:1].broadcast_to((P, cw)),
                    dst,
                    initial,
                    Alu.mult,
                    Alu.add,
                )
                if t >= n_burn:
                    i = t - n_burn
                    nc.scalar.activation(
                        scratch_sq[:, :cw],
                        dst,
                        Act.Square,
                        accum_out=qcol[:, i : i + 1],
                    )
                    if t == last:
                        # keep the tail short: last chunk's sum on DVE,
                        # parallel with ScalarE's Square.
                        nc.vector.tensor_scalar(
                            scratch_dve[:, :cw],
                            dst,
                            1.0,
                            None,
                            Alu.mult,
                            Alu.add,
                            accum_out=scol[:, i : i + 1],
                        )
                    else:
                        nc.scalar.activation(
                            scratch_sum[:, :cw],
                            dst,
                            Act.Copy,
                            accum_out=scol[:, i : i + 1],
                        )

            # ---- local totals -> [1,2] -> AllGather -> [8,2] ----
            nc.vector.tensor_reduce(pay[:, 0:1], scol[:, :], Axis.X, Alu.add)
            nc.vector.tensor_reduce(pay[:, 1:2], qcol[:, :], Axis.X, Alu.add)
            nc.gpsimd.tensor_reduce(cc_sb[0:1, 0:1], pay[:, 0:1], Axis.C, Alu.add)
            nc.gpsimd.tensor_reduce(cc_sb[0:1, 1:2], pay[:, 1:2], Axis.C, Alu.add)
            nc.scalar.dma_start(cc_in.ap(), cc_sb[0:1, :])
            nc.gpsimd.collective_compute(
                "AllGather",
                Alu.bypass,
                replica_groups=rg,
                ins=[cc_in.ap().opt()],
                outs=[cc_out.ap().opt()],
            )
            # broadcast the 16 gathered floats to every partition
            gsrc = bass.AP(cc_out, 0, [[0, P], [1, 2 * N_CORES]])
            nc.scalar.dma_start(gath[:, :], gsrc)

            # ---- global stats (every partition, redundantly) ----
            # m = [sum, sumsq] / T; negvar = mean^2 - msq; std = sqrt(-negvar)
            nc.vector.tensor_reduce(
                m2[:, :].unsqueeze(2),
                gath[:, :].rearrange("p (r j) -> p j r", j=2),
                Axis.X,
                Alu.add,
            )
            nc.vector.tensor_scalar(m2[:, :], m2[:, :], inv_T, None, Alu.mult)
            nc.vector.scalar_tensor_tensor(
                var[:, :], m2[:, 0:1], m2[:, 0:1], m2[:, 1:2], Alu.mult, Alu.subtract
            )
            nc.scalar.activation(std[:, :], var[:, :], Act.Sqrt, scale=-1.0)
            nc.vector.tensor_scalar(std[:, :], std[:, :], EPS, None, Alu.add)
            nc.vector.reciprocal(inv[:, :], std[:, :])
            nc.vector.tensor_tensor(shiftp[:, :], m2[:, 0:1], inv[:, :], Alu.mult)

            # ---- normalize in place + paired DMA out (alternating rings) ----
            pend = None  # (start_col, width) of normalized-but-unsent region
            k = 0
            for t, (c0, cw) in enumerate(valid_chunks):
                seg = resident[:, c0 : c0 + cw]
                # out = y*inv - mean*inv
                nc.vector.tensor_scalar(
                    seg, seg, inv[:, 0:1], shiftp[:, 0:1], Alu.mult, Alu.subtract
                )
                if pend is None and t < len(valid_chunks) - 1:
                    pend = (c0, cw)
                    continue
                o0, ow = (pend[0], pend[1] + cw) if pend else (c0, cw)
                pend = None
                dst = bass.AP(out_ext, o0 - W, [[L, P], [1, ow]])
                eng = nc.sync if k % 2 == 0 else nc.scalar
                k += 1
                eng.dma_start(dst, resident[:, o0 : o0 + ow])

    nc.compile()
    return nc


_CACHED = {}


def _get_nc(C, W, F):
    key = (C, W, F)
    if key not in _CACHED:
        _CACHED[key] = _build_nc(C, W, F)
    return _CACHED[key]


def run_sharded(rewards, C=None, W=1024, F=4096, **spmd_kwargs):
    """Shard, run on 8 cores, gather. Returns (output, BassKernelResults)."""
    from concourse import bass_utils

    r = np.ascontiguousarray(np.asarray(rewards, dtype=np.float32))
    total = r.shape[0]
    if C is None:
        C = total // N_CORES
    assert C * N_CORES == total

    nc = _get_nc(C, W, F)

    s_pad = np.empty(total + W, dtype=np.float32)
    s_pad[:W] = 0.0
    s_pad[W:] = r[::-1]
    in_maps = [
        {"s": np.ascontiguousarray(s_pad[c * C : (c + 1) * C + W])}
        for c in range(N_CORES)
    ]
    res = bass_utils.run_bass_kernel_spmd(
        nc, in_maps, core_ids=list(range(N_CORES)), **spmd_kwargs
    )
    y = np.concatenate([res.results[c]["out"].reshape(-1) for c in range(N_CORES)])
    return np.ascontiguousarray(y[::-1]), res


def kernel(rewards):
    out, _ = run_sharded(rewards)
    return out
